# revision 7
# baseline (speedup 1.0000x reference)
"""LightGCN encoder on 8 Trainium2 NeuronCores.

Row-parallel SpMM: nodes (segment-sum destinations) are sharded across the 8
cores; edges are partitioned by destination row. Per layer each core gathers
source embeddings with dma_gather (SWDGE ucode), folds edge values into a
one-hot matrix built on DVE (tensor_scalar is_equal*mult against an iota), and
segment-sums via PE matmuls accumulating in PSUM. Updated shards are exchanged
with AllGather collectives between the S-graph and adjacency hops.

Host-side preprocessing sorts/pads the edge lists per (dest-block,
source-range) — identical segment layout on every core so one SPMD program
serves all 8 — and maps source ids into the padded table coordinates used on
device. int16 gather indices limit a single gather table to 32K rows, hence
the 4 (adj) / 2 (S) source ranges of 25088 padded rows each.
"""

from contextlib import ExitStack

import numpy as np

NC = 8
EMB = 64
U = 50000
NNODES = 100000
NLAYERS = 3

ASHARD = 12500            # adj dest rows per core
ABLOCKS = 98              # ceil(12500/128)
APAD = ABLOCKS * 128      # 12544
SSHARD = 6250             # user dest rows per core
SBLOCKS = 49
SPAD = SBLOCKS * 128      # 6272
ETAB = NC * APAD          # 100352 padded ego table rows
UTAB = NC * SPAD          # 50176 padded user table rows
RNG = 25088               # rows per gather range (int16-safe, = ETAB/4 = UTAB/2)
SBSZ = 8                  # dest blocks per superblock (one gather call per range)

_cache = {}


def _pad_coords_ego(col):
    # global node id -> row in the 8x12544-padded ego table
    return (col // ASHARD) * APAD + col % ASHARD


def _pad_coords_u(col):
    # global user id -> row in the 8x6272-padded user table
    return (col // SSHARD) * SPAD + col % SSHARD


def _layout(seg_lens, nblocks, nranges):
    """Edge ordering: superblock -> range -> block. Returns (off[b][r], sb
    descriptors, total). All lengths are multiples of 128."""
    off = np.zeros((nblocks, nranges), np.int64)
    sbs = []
    pos = 0
    for sb0 in range(0, nblocks, SBSZ):
        blocks = list(range(sb0, min(sb0 + SBSZ, nblocks)))
        sb_start = pos
        rng_info = []
        for r in range(nranges):
            r_start = pos
            for b in blocks:
                off[b, r] = pos
                pos += int(seg_lens[b, r])
            rng_info.append((r_start, pos - r_start))
        sbs.append({"blocks": blocks, "start": sb_start, "end": pos,
                    "ranges": rng_info})
    return off, sbs, pos


def _group_and_fill(block, rng_id, local, within, val, nranges, seg_lens, off,
                    total):
    """Place edges (any order) into the padded global layout. Padding slots
    keep idx=0/dest=0/val=0 (gather row 0 of the range, scaled by 0)."""
    idx = np.zeros(total, np.int16)
    dest = np.zeros(total, np.float32)
    v = np.zeros(total, np.float32)
    grp = block.astype(np.int64) * nranges + rng_id
    order = np.argsort(grp, kind="stable")
    g_sorted = grp[order]
    uniq, starts = np.unique(g_sorted, return_index=True)
    counts = np.diff(np.append(starts, len(order)))
    base_of = np.repeat(off.reshape(-1)[uniq], counts)
    grp_start_of = np.repeat(starts, counts)
    pos_sorted = base_of + (np.arange(len(order)) - grp_start_of)
    idx[pos_sorted] = local[order].astype(np.int16)
    dest[pos_sorted] = within[order].astype(np.float32)
    v[pos_sorted] = val[order]
    return idx, dest, v


def _wrap16(a):
    w = np.ascontiguousarray(a.reshape(-1, 16).T)  # [16, E/16]
    return np.tile(w, (8, 1))                      # replicated for 8 Q7 cores


def _wrap128(a):
    return np.ascontiguousarray(a.reshape(-1, 128).T)


def _preprocess(user_emb, item_emb, adj_rows, adj_cols, adj_vals, s_rows,
                s_cols, s_vals):
    ego0 = np.concatenate([np.asarray(user_emb), np.asarray(item_emb)], axis=0)
    ego0_pad = np.zeros((ETAB, EMB), np.float32)
    for c in range(NC):
        ego0_pad[c * APAD:c * APAD + ASHARD] = ego0[c * ASHARD:(c + 1) * ASHARD]

    adj_rows = np.asarray(adj_rows).astype(np.int64)
    adj_cols = np.asarray(adj_cols).astype(np.int64)
    adj_vals = np.asarray(adj_vals).astype(np.float32)
    s_rows_l = np.asarray(s_rows).astype(np.int64)
    s_cols_l = np.asarray(s_cols).astype(np.int64)
    s_vals_l = np.asarray(s_vals).astype(np.float32)

    # S' = I + S (self edges make u_new = u + S@u a pure segment-sum)
    self_dest = np.arange(U, dtype=np.int64)
    s_rows_l = np.concatenate([s_rows_l, self_dest])
    s_cols_l = np.concatenate([s_cols_l, self_dest])
    s_vals_l = np.concatenate([s_vals_l, np.ones(U, np.float32)])

    per_core = []
    for c in range(NC):
        m = (adj_rows >= c * ASHARD) & (adj_rows < (c + 1) * ASHARD)
        d = adj_rows[m] - c * ASHARD
        col = adj_cols[m]
        # adj sources: users -> u-table coords (ranges 0,1), items -> ego
        # table item half (ranges 2,3)
        is_u = col < U
        up = _pad_coords_u(np.where(is_u, col, 0))
        ep = _pad_coords_ego(np.where(is_u, 0, col)) - UTAB  # item half offset
        rng_id = np.where(is_u, up // RNG, 2 + ep // RNG)
        local = np.where(is_u, up % RNG, ep % RNG)
        a = dict(block=d // 128, rng=rng_id, local=local, within=d % 128,
                 val=adj_vals[m])

        m = (s_rows_l >= c * SSHARD) & (s_rows_l < (c + 1) * SSHARD)
        d = s_rows_l[m] - c * SSHARD
        sp = _pad_coords_ego(s_cols_l[m])  # users live in ego table rows <UTAB
        s = dict(block=d // 128, rng=sp // RNG, local=sp % RNG, within=d % 128,
                 val=s_vals_l[m])
        per_core.append((a, s))

    def seg_max(key, nblocks, nranges):
        lens = np.zeros((nblocks, nranges), np.int64)
        for c in range(NC):
            e = per_core[c][0 if key == "a" else 1]
            cnt = np.bincount(e["block"] * nranges + e["rng"],
                              minlength=nblocks * nranges)
            lens = np.maximum(lens, cnt.reshape(nblocks, nranges))
        return ((lens + 127) // 128) * 128

    a_lens = seg_max("a", ABLOCKS, 4)
    s_lens = seg_max("s", SBLOCKS, 2)
    a_off, a_sbs, a_total = _layout(a_lens, ABLOCKS, 4)
    s_off, s_sbs, s_total = _layout(s_lens, SBLOCKS, 2)

    in_maps = []
    for c in range(NC):
        a, s = per_core[c]
        aidx, adest, aval = _group_and_fill(a["block"], a["rng"], a["local"],
                                            a["within"], a["val"], 4, a_lens,
                                            a_off, a_total)
        sidx, sdest, sval = _group_and_fill(s["block"], s["rng"], s["local"],
                                            s["within"], s["val"], 2, s_lens,
                                            s_off, s_total)
        in_maps.append({
            "ego0": ego0_pad,
            "acc0": np.ascontiguousarray(ego0_pad[c * APAD:(c + 1) * APAD]),
            "aidx": _wrap16(aidx), "adest": _wrap128(adest),
            "aval": _wrap128(aval),
            "sidx": _wrap16(sidx), "sdest": _wrap128(sdest),
            "sval": _wrap128(sval),
            "iota": np.broadcast_to(np.arange(128, dtype=np.float32),
                                    (128, 128)).copy(),
        })
    meta = dict(a_lens=a_lens, s_lens=s_lens, a_off=a_off, s_off=s_off,
                a_sbs=a_sbs, s_sbs=s_sbs, a_total=a_total, s_total=s_total)
    return in_maps, meta


def _build(meta):
    import concourse.tile as tile
    from concourse import bacc, mybir

    f32 = mybir.dt.float32
    i16 = mybir.dt.int16
    nc = bacc.Bacc("TRN2", target_bir_lowering=False, debug=False,
                   num_devices=NC, num_swdge_queues=4)

    ego0_d = nc.dram_tensor("ego0", [ETAB, EMB], f32, kind="ExternalInput")
    acc0_d = nc.dram_tensor("acc0", [APAD, EMB], f32, kind="ExternalInput")
    aidx_d = nc.dram_tensor("aidx", [128, meta["a_total"] // 16], i16,
                            kind="ExternalInput")
    adest_d = nc.dram_tensor("adest", [128, meta["a_total"] // 128], f32,
                             kind="ExternalInput")
    aval_d = nc.dram_tensor("aval", [128, meta["a_total"] // 128], f32,
                            kind="ExternalInput")
    sidx_d = nc.dram_tensor("sidx", [128, meta["s_total"] // 16], i16,
                            kind="ExternalInput")
    sdest_d = nc.dram_tensor("sdest", [128, meta["s_total"] // 128], f32,
                             kind="ExternalInput")
    sval_d = nc.dram_tensor("sval", [128, meta["s_total"] // 128], f32,
                            kind="ExternalInput")
    iota_d = nc.dram_tensor("iota", [128, 128], f32, kind="ExternalInput")
    out_d = nc.dram_tensor("out", [APAD, EMB], f32, kind="ExternalOutput")

    with tile.TileContext(nc) as tc, ExitStack() as ctx:
        persist = ctx.enter_context(tc.tile_pool(name="persist", bufs=1))
        metap = ctx.enter_context(tc.tile_pool(name="meta", bufs=2))
        gpool = ctx.enter_context(tc.tile_pool(name="g", bufs=2))
        ohp = ctx.enter_context(tc.tile_pool(name="oh", bufs=4))
        evp = ctx.enter_context(tc.tile_pool(name="ev", bufs=4))
        psp = ctx.enter_context(tc.tile_pool(name="ps", bufs=8, space="PSUM"))
        dram = ctx.enter_context(tc.tile_pool(name="dr", bufs=1, space="DRAM"))

        iota_t = persist.tile([128, 128], f32, tag="iota")
        nc.sync.dma_start(out=iota_t[:], in_=iota_d.ap())
        acc_t = persist.tile([128, ABLOCKS, EMB], f32, tag="acc")
        nc.sync.dma_start(
            out=acc_t[:],
            in_=acc0_d.ap().rearrange("(b p) e -> p b e", p=128))

        agu_in = [dram.tile([SPAD, EMB], f32, tag=f"agui{l}", name=f"agui{l}")
                  for l in range(NLAYERS)]
        agu_out = [dram.tile([UTAB, EMB], f32, tag=f"aguo{l}", name=f"aguo{l}",
                             addr_space="Shared") for l in range(NLAYERS)]
        agego_in = [dram.tile([APAD, EMB], f32, tag=f"agei{l}",
                              name=f"agei{l}") for l in range(NLAYERS - 1)]
        agego_out = [dram.tile([ETAB, EMB], f32, tag=f"ageo{l}",
                               name=f"ageo{l}", addr_space="Shared")
                     for l in range(NLAYERS - 1)]

        self_q = [0]  # round-robin SWDGE queue counter

        def do_phase(sbs, lens, off, tabs, idx_d, dest_d, val_d, out_dram,
                     use_acc):
            nranges = len(tabs)
            for sb in sbs:
                e0, e1 = sb["start"], sb["end"]
                idx_t = metap.tile([128, (e1 - e0) // 16], i16, tag="idx", name="idx_t")
                nc.sync.dma_start(out=idx_t[:],
                                  in_=idx_d.ap()[:, e0 // 16:e1 // 16])
                dest_t = metap.tile([128, (e1 - e0) // 128], f32, tag="dest", name="dest_t")
                nc.sync.dma_start(out=dest_t[:],
                                  in_=dest_d.ap()[:, e0 // 128:e1 // 128])
                val_t = metap.tile([128, (e1 - e0) // 128], f32, tag="val", name="val_t")
                nc.sync.dma_start(out=val_t[:],
                                  in_=val_d.ap()[:, e0 // 128:e1 // 128])
                g = {}
                for r in range(nranges):
                    r0, rn = sb["ranges"][r]
                    if rn == 0:
                        continue
                    g[r] = gpool.tile([128, rn // 128, EMB], f32, tag=f"g{r}", name=f"g{r}")
                    # dma_gather ucode caps at 1024 indices per call; chunk
                    # and spread over the 4 SWDGE queues
                    for c0 in range(0, rn, 1024):
                        n = min(1024, rn - c0)
                        a0, a1 = r0 + c0, r0 + c0 + n
                        nc.gpsimd.dma_gather(
                            out_ap=g[r][:, c0 // 128:(c0 + n) // 128, :],
                            in_ap=tabs[r],
                            idxs_ap=idx_t[:, (a0 - e0) // 16:(a1 - e0) // 16],
                            num_idxs=n, num_idxs_reg=n, elem_size=EMB,
                            queue_num=self_q[0] % 4)
                        self_q[0] += 1
                for b in sb["blocks"]:
                    ntiles = int(sum(lens[b])) // 128
                    if ntiles == 0:
                        continue
                    ps = psp.tile([128, EMB], f32, name="ps")
                    ti = 0
                    for r in range(nranges):
                        r0, _ = sb["ranges"][r]
                        for t in range(int(lens[b][r]) // 128):
                            col = (int(off[b][r]) + t * 128 - e0) // 128
                            gcol = (int(off[b][r]) + t * 128 - r0) // 128
                            oh = ohp.tile([128, 128], f32, tag="oh", name="oh")
                            nc.vector.tensor_scalar(
                                oh[:], iota_t[:], dest_t[:, col:col + 1],
                                val_t[:, col:col + 1],
                                mybir.AluOpType.is_equal,
                                mybir.AluOpType.mult)
                            nc.tensor.matmul(ps[:], lhsT=oh[:],
                                             rhs=g[r][:, gcol, :],
                                             start=(ti == 0),
                                             stop=(ti == ntiles - 1))
                            ti += 1
                    if out_dram is not None:
                        ev = evp.tile([128, EMB], f32, tag="ev", name="ev")
                        nc.vector.tensor_copy(ev[:], ps[:])
                        nc.sync.dma_start(
                            out=out_dram[b * 128:(b + 1) * 128, :], in_=ev[:])
                        if use_acc:
                            nc.vector.tensor_add(acc_t[:, b, :],
                                                 acc_t[:, b, :], ev[:])
                    elif use_acc:
                        nc.vector.tensor_add(acc_t[:, b, :], acc_t[:, b, :],
                                             ps[:])

        for l in range(NLAYERS):
            ego_tab = ego0_d.ap() if l == 0 else agego_out[l - 1][:]
            # S hop: u' = (I+S) @ u over the user half of the ego table
            do_phase(meta["s_sbs"], meta["s_lens"], meta["s_off"],
                     [ego_tab[0:RNG], ego_tab[RNG:2 * RNG]],
                     sidx_d, sdest_d, sval_d, agu_in[l][:], False)
            nc.gpsimd.collective_compute(
                "AllGather", mybir.AluOpType.bypass,
                replica_groups=[list(range(NC))],
                ins=[agu_in[l][:].opt()], outs=[agu_out[l][:].opt()])
            # adjacency hop over [u'; v]
            tabs = [agu_out[l][:][0:RNG], agu_out[l][:][RNG:2 * RNG],
                    ego_tab[2 * RNG:3 * RNG], ego_tab[3 * RNG:4 * RNG]]
            last = l == NLAYERS - 1
            do_phase(meta["a_sbs"], meta["a_lens"], meta["a_off"], tabs,
                     aidx_d, adest_d, aval_d,
                     None if last else agego_in[l][:], True)
            if not last:
                nc.gpsimd.collective_compute(
                    "AllGather", mybir.AluOpType.bypass,
                    replica_groups=[list(range(NC))],
                    ins=[agego_in[l][:].opt()], outs=[agego_out[l][:].opt()])

        for b in range(ABLOCKS):
            nc.vector.tensor_scalar_mul(acc_t[:, b, :], acc_t[:, b, :],
                                        1.0 / (NLAYERS + 1))
        nc.sync.dma_start(
            out=out_d.ap().rearrange("(b p) e -> p b e", p=128), in_=acc_t[:])

    nc.compile()
    return nc


def kernel(user_emb, item_emb, adj_rows, adj_cols, adj_vals, s_rows, s_cols,
           s_vals):
    from concourse.bass_utils import run_bass_kernel_spmd

    in_maps, meta = _preprocess(user_emb, item_emb, adj_rows, adj_cols,
                                adj_vals, s_rows, s_cols, s_vals)
    key = (meta["a_total"], meta["s_total"])
    if _cache.get("key") != key:
        _cache["nc"] = _build(meta)
        _cache["key"] = key
    res = run_bass_kernel_spmd(_cache["nc"], in_maps,
                               core_ids=list(range(NC)))
    _cache["last_results"] = res
    full = np.empty((NNODES, EMB), np.float32)
    for c in range(NC):
        full[c * ASHARD:(c + 1) * ASHARD] = res.results[c]["out"][:ASHARD]
    return full[:U], full[U:]
